# revision 1
# baseline (speedup 1.0000x reference)
"""GAT (2-layer, 8-head) forward on 8 Trainium2 NeuronCores via Bass/Tile.

Strategy (per sharding hint): partition nodes across 8 cores; each core owns the
edges whose destination lands in its partition, so segment-softmax/aggregation
are local. Within a core, destination nodes are bin-packed into 49 windows of
128 nodes; each window's incoming edges occupy <= 19 subtiles of 128 edge slots.
Per edge subtile: indirect-DMA row gathers fetch source features, attention
weights are computed on-chip, and a one-hot [edge x dst] matrix on the tensor
engine performs the segment-sum scatter (messages + softmax denominator in the
same PSUM accumulation). Layer 2 is transform-then-aggregate: z = elu(out1)@W2
is computed once per node, all-gathered across cores, and layer-2 attention
aggregates 33-float z rows. log_softmax is fused into the layer-2 finalize.
"""
import sys

sys.path.insert(0, "/opt/trn_rl_repo")

import numpy as np
from contextlib import ExitStack

import concourse.bass as bass
import concourse.tile as tile
from concourse import bacc, mybir
from concourse.bass_utils import run_bass_kernel_spmd

F32 = mybir.dt.float32
F32R = mybir.dt.float32r
I32 = mybir.dt.int32
AF = mybir.ActivationFunctionType
OP = mybir.AluOpType

# problem constants (hardcoded per contract)
N = 50000
E = 800000
IN_C = 128
HID = 32
HEADS = 8
OUT_C = 32
NEG = 0.2

NCORES = 8
NODES_PC = N // NCORES      # 6250
NW = 49                     # windows per core
WSLOT = 128
KSUB = 19                   # edge subtiles per window
CAP = KSUB * 128            # 2432
GPC = NW * WSLOT            # 6272
TOT = NCORES * GPC          # 50176
NCOL = NW * KSUB            # 931
NT0 = TOT // 128            # 392 phase-0 tiles
PAD_DST = 200.0


# ----------------------------------------------------------------------------
# host preprocessing
# ----------------------------------------------------------------------------

def _preprocess(edge_index):
    src = np.concatenate([edge_index[0], np.arange(N, dtype=np.int64)])
    dst = np.concatenate([edge_index[1], np.arange(N, dtype=np.int64)])
    Etot = src.shape[0]

    deg = np.bincount(dst, minlength=N)
    node_bin = np.zeros(N, dtype=np.int32)
    node_slot = np.zeros(N, dtype=np.int32)
    for c in range(NCORES):
        nodes = np.arange(c * NODES_PC, (c + 1) * NODES_PC)
        order = np.argsort(-deg[nodes], kind="stable")
        bins_edges = np.zeros(NW, dtype=np.int64)
        bins_count = np.zeros(NW, dtype=np.int64)
        for n in nodes[order]:
            d = deg[n]
            feas = (bins_count < WSLOT) & (bins_edges + d <= CAP)
            assert feas.any(), "window capacity overflow"
            b = int(np.argmin(np.where(feas, bins_edges, np.iinfo(np.int64).max)))
            node_bin[n] = b
            node_slot[n] = bins_count[b]
            bins_edges[b] += d
            bins_count[b] += 1

    node_gid = (np.arange(N) // NODES_PC) * GPC + node_bin * WSLOT + node_slot

    ecore = (dst // NODES_PC).astype(np.int64)
    ebin = node_bin[dst].astype(np.int64)
    key = ecore * NW + ebin
    eorder = np.argsort(key, kind="stable")
    key_sorted = key[eorder]
    grp_start = np.searchsorted(key_sorted, np.arange(NCORES * NW), side="left")
    pos_in_grp = np.arange(Etot) - grp_start[key_sorted]
    eslot = np.empty(Etot, dtype=np.int64)
    eslot[eorder] = pos_in_grp
    assert (eslot < CAP).all()

    src_idx = np.zeros((NCORES, 128, NCOL), dtype=np.int32)
    dstg_idx = np.zeros((NCORES, 128, NCOL), dtype=np.int32)
    dst_col = np.full((NCORES, 128, NCOL), PAD_DST, dtype=np.float32)
    p = (eslot % 128).astype(np.int64)
    colj = (ebin * KSUB + eslot // 128).astype(np.int64)
    src_idx[ecore, p, colj] = node_gid[src]
    dstg_idx[ecore, p, colj] = node_gid[dst]
    dst_col[ecore, p, colj] = node_slot[dst]

    return src_idx, dstg_idx, dst_col, node_gid


# ----------------------------------------------------------------------------
# bass program
# ----------------------------------------------------------------------------

def _build_program(timing=False, phases=(1, 1, 1)):
    nc = bacc.Bacc("TRN2", target_bir_lowering=False, debug=False,
                   num_devices=NCORES, num_swdge_queues=4)

    _gq = [0]

    def _gather(out_ap, table, idx_ap, element_offset=0):
        bi = nc.gpsimd.indirect_dma_start(
            out=out_ap, out_offset=None, in_=table,
            in_offset=bass.IndirectOffsetOnAxis(ap=idx_ap, axis=0),
            element_offset=element_offset)
        return bi

    xTw_d = nc.dram_tensor("xTw", [IN_C, TOT], F32R, kind="ExternalInput").ap()
    w1a_d = nc.dram_tensor("w1a", [IN_C, 272], F32R, kind="ExternalInput").ap()
    w2a_d = nc.dram_tensor("w2a", [128, 68], F32R, kind="ExternalInput").ap()
    iota_d = nc.dram_tensor("iota", [128, 128], F32, kind="ExternalInput").ap()
    ident_d = nc.dram_tensor("ident", [128, 128], F32, kind="ExternalInput").ap()
    b1t_d = nc.dram_tensor("b1t", [128, 256], F32, kind="ExternalInput").ap()
    b2t_d = nc.dram_tensor("b2t", [128, 32], F32, kind="ExternalInput").ap()
    sidx_d = nc.dram_tensor("sidx", [128, NCOL], I32, kind="ExternalInput").ap()
    didx_d = nc.dram_tensor("didx", [128, NCOL], I32, kind="ExternalInput").ap()
    dcol_d = nc.dram_tensor("dcol", [128, NCOL], F32, kind="ExternalInput").ap()

    out_d = nc.dram_tensor("out2", [GPC, OUT_C], F32, kind="ExternalOutput").ap()

    h1tab = nc.dram_tensor("h1tab", [TOT, 264], F32R, kind="Internal").ap()
    sdtab = nc.dram_tensor("sdtab", [TOT, 8], F32, kind="Internal").ap()
    zz_own = nc.dram_tensor("zz_own", [GPC, 34], F32R, kind="Internal").ap()
    zz_all = nc.dram_tensor("zz_all", [TOT, 34], F32R, kind="Internal",
                            addr_space="Shared").ap()

    with tile.TileContext(nc) as tc, ExitStack() as ctx:
        cons = ctx.enter_context(tc.tile_pool(name="cons", bufs=1))
        stat = ctx.enter_context(tc.tile_pool(name="stat", bufs=3))
        gath = ctx.enter_context(tc.tile_pool(name="gath", bufs=2))
        work = ctx.enter_context(tc.tile_pool(name="work", bufs=3))
        sub = ctx.enter_context(tc.tile_pool(name="sub", bufs=4))
        pp = ctx.enter_context(tc.tile_pool(name="pp", bufs=2, space="PSUM"))

        # ---- constants resident in SBUF ----
        w1a_t = cons.tile([IN_C, 272], F32R)
        nc.sync.dma_start(w1a_t[:], w1a_d)
        w2a_t = cons.tile([128, 68], F32R)
        nc.sync.dma_start(w2a_t[:], w2a_d)
        iota_t = cons.tile([128, 128], F32)
        nc.sync.dma_start(iota_t[:], iota_d)
        ident_t = cons.tile([128, 128], F32)
        nc.sync.dma_start(ident_t[:], ident_d)
        b1t_t = cons.tile([128, 256], F32)
        nc.sync.dma_start(b1t_t[:], b1t_d)
        b2t_t = cons.tile([128, 32], F32)
        nc.sync.dma_start(b2t_t[:], b2t_d)
        sidx_t = cons.tile([128, NCOL], I32)
        nc.sync.dma_start(sidx_t[:], sidx_d)
        didx_t = cons.tile([128, NCOL], I32)
        nc.sync.dma_start(didx_t[:], didx_d)
        dcol_t = cons.tile([128, NCOL], F32)
        nc.sync.dma_start(dcol_t[:], dcol_d)

        # ---- phase 0: h1 table [TOT, 264] + s_dst table [TOT, 8] ----
        for t in range(NT0 if phases[0] else 0):
            xt = stat.tile([IN_C, 128], F32R, tag="xt")
            nc.sync.dma_start(xt[:], xTw_d[:, t * 128:(t + 1) * 128])
            ph = pp.tile([128, 272], F32, tag="ph0")
            nc.tensor.matmul(ph[:], xt[:], w1a_t[:], start=True, stop=True)
            stg = work.tile([128, 272], F32R, tag="stg0")
            nc.scalar.activation(stg[:], ph[:], AF.Copy)
            nc.sync.dma_start(h1tab[t * 128:(t + 1) * 128, :], stg[:, 0:264])
            nc.sync.dma_start(sdtab[t * 128:(t + 1) * 128, :],
                              stg[:, 264:272].bitcast(F32))

        # ---- layer 1 ----
        for w in range(NW if phases[1] else 0):
            g_t = gath.tile([128, KSUB, 264], F32R, tag="g1")
            sd_t = gath.tile([128, KSUB, 8], F32, tag="sd1")
            for j in range(KSUB):
                col = w * KSUB + j
                _gather(g_t[:, j, :], h1tab, sidx_t[:, col:col + 1])
                _gather(sd_t[:, j, :], sdtab, didx_t[:, col:col + 1])
            # scores (bulk per window): e = leaky(s_src + s_dst); ex = exp(e)
            e_t = work.tile([128, KSUB, 8], F32, tag="e1")
            nc.vector.tensor_tensor(e_t[:], g_t[:, :, 256:264].bitcast(F32),
                                    sd_t[:], OP.add)
            lk_t = work.tile([128, KSUB, 8], F32, tag="lk1")
            nc.vector.scalar_tensor_tensor(lk_t[:], e_t[:], NEG, e_t[:],
                                           OP.mult, OP.max)
            msg_t = work.tile([128, KSUB, 264], F32R, tag="msg1")
            nc.scalar.activation(msg_t[:, :, 256:264], lk_t[:], AF.Exp)
            # messages (bulk): msg = h * ex  (per-head broadcast)
            nc.vector.tensor_tensor(
                msg_t[:, :, 0:256].rearrange("p k (h c) -> p k h c", h=HEADS),
                g_t[:, :, 0:256].bitcast(F32).rearrange(
                    "p k (h c) -> p k h c", h=HEADS),
                msg_t[:, :, 256:264].bitcast(F32).unsqueeze(3).broadcast_to(
                    [128, KSUB, HEADS, HID]),
                OP.mult)
            # scatter: one-hot matmuls accumulate into window PSUM
            acc = pp.tile([128, 264], F32, tag="acc")
            for j in range(KSUB):
                col = w * KSUB + j
                s_t = sub.tile([128, 128], F32R, tag="s1")
                nc.vector.tensor_scalar(s_t[:], iota_t[:],
                                        dcol_t[:, col:col + 1], None,
                                        OP.is_equal)
                nc.tensor.matmul(acc[:], s_t[:], msg_t[:, j, :],
                                 start=(j == 0), stop=(j == KSUB - 1))
            # finalize window: out1 = acc/den + b1; h2 = elu(out1)
            denc = work.tile([128, 8], F32, tag="denc")
            nc.vector.tensor_scalar(denc[:], acc[:, 256:264], 1e-30, None,
                                    OP.max)
            rden = work.tile([128, 8], F32, tag="rden")
            nc.vector.reciprocal(rden[:], denc[:])
            o1 = work.tile([128, 256], F32, tag="o1")
            nc.vector.tensor_tensor(
                o1[:].rearrange("p (h c) -> p h c", h=HEADS),
                acc[:, 0:256].rearrange("p (h c) -> p h c", h=HEADS),
                rden[:].unsqueeze(2).broadcast_to([128, HEADS, HID]),
                OP.mult)
            h2a = work.tile([128, 256], F32, tag="h2a")
            nc.vector.tensor_tensor(h2a[:], o1[:], b1t_t[:], OP.add)
            tmin = work.tile([128, 256], F32, tag="tmin")
            nc.vector.tensor_scalar(tmin[:], h2a[:], 0.0, None, OP.min)
            eexp = work.tile([128, 256], F32, tag="eexp")
            nc.scalar.activation(eexp[:], tmin[:], AF.Exp)
            rl = work.tile([128, 256], F32, tag="rl")
            nc.vector.tensor_scalar(rl[:], h2a[:], 0.0, None, OP.max)
            h2e = work.tile([128, 256], F32, tag="h2e")
            nc.vector.scalar_tensor_tensor(h2e[:], eexp[:], -1.0, rl[:],
                                           OP.add, OP.add)
            # z = h2e @ W2 (+ attention vectors) via transpose + 2 matmuls
            zps = pp.tile([128, 34], F32, tag="zps")
            for half in range(2):
                trp = pp.tile([128, 128], F32, tag="trp")
                nc.tensor.transpose(trp[:], h2e[:, half * 128:(half + 1) * 128],
                                    ident_t[:])
                h2T = sub.tile([128, 128], F32R, tag="h2T")
                nc.scalar.activation(h2T[:], trp[:], AF.Copy)
                nc.tensor.matmul(zps[:], h2T[:],
                                 w2a_t[:, half * 34:(half + 1) * 34],
                                 start=(half == 0), stop=(half == 1))
            zst = work.tile([128, 34], F32R, tag="zst")
            nc.scalar.activation(zst[:], zps[:], AF.Copy)
            nc.sync.dma_start(zz_own[w * 128:(w + 1) * 128, :], zst[:])

        # ---- all-gather z across cores ----
        if timing:
            # TimelineSim can't model collectives; stand in the same bytes
            # (each core receives NCORES slices) with plain DMAs.
            for c in range(NCORES):
                nc.sync.dma_start(zz_all[c * GPC:(c + 1) * GPC, :], zz_own)
        else:
            nc.gpsimd.collective_compute(
                "AllGather", OP.bypass,
                replica_groups=[list(range(NCORES))],
                ins=[zz_own], outs=[zz_all])

        # ---- layer 2 ----
        for w in range(NW if phases[2] else 0):
            gz_t = gath.tile([128, KSUB, 34], F32R, tag="g2")
            sd2_t = gath.tile([128, KSUB], F32, tag="sd2")
            for j in range(KSUB):
                col = w * KSUB + j
                _gather(gz_t[:, j, :], zz_all, sidx_t[:, col:col + 1])
                _gather(sd2_t[:, j:j + 1].bitcast(F32R), zz_all,
                        didx_t[:, col:col + 1], element_offset=33)
            e2_t = work.tile([128, KSUB], F32, tag="e2")
            nc.vector.tensor_tensor(e2_t[:],
                                    gz_t[:, :, 32:33].bitcast(F32).squeeze(2),
                                    sd2_t[:], OP.add)
            lk2_t = work.tile([128, KSUB], F32, tag="lk2")
            nc.vector.scalar_tensor_tensor(lk2_t[:], e2_t[:], NEG, e2_t[:],
                                           OP.mult, OP.max)
            ex2_t = work.tile([128, KSUB], F32R, tag="ex2")
            nc.scalar.activation(ex2_t[:], lk2_t[:], AF.Exp)
            nc.vector.memset(gz_t[:, :, 32:33].bitcast(F32), 1.0)

            acc2 = pp.tile([128, 34], F32, tag="acc")
            for j in range(KSUB):
                col = w * KSUB + j
                s_t = sub.tile([128, 128], F32R, tag="s2")
                nc.vector.tensor_scalar(s_t[:], iota_t[:],
                                        dcol_t[:, col:col + 1], None,
                                        OP.is_equal)
                gzs = sub.tile([128, 34], F32R, tag="gzs")
                nc.vector.tensor_scalar(gzs[:], gz_t[:, j, 0:34].bitcast(F32),
                                        ex2_t[:, j:j + 1].bitcast(F32), None,
                                        OP.mult)
                nc.tensor.matmul(acc2[:], s_t[:], gzs[:],
                                 start=(j == 0), stop=(j == KSUB - 1))
            # finalize: out2 = log_softmax(acc2/den + b2)
            den2 = work.tile([128, 1], F32, tag="den2")
            nc.vector.tensor_scalar(den2[:], acc2[:, 32:33], 1e-30, None,
                                    OP.max)
            rd2 = work.tile([128, 1], F32, tag="rd2")
            nc.vector.reciprocal(rd2[:], den2[:])
            o2 = work.tile([128, 32], F32, tag="o2")
            nc.vector.tensor_scalar(o2[:], acc2[:, 0:32], rd2[:], None, OP.mult)
            o2b = work.tile([128, 32], F32, tag="o2b")
            nc.vector.tensor_tensor(o2b[:], o2[:], b2t_t[:], OP.add)
            mx = work.tile([128, 1], F32, tag="mx")
            nc.vector.tensor_reduce(mx[:], o2b[:], mybir.AxisListType.X, OP.max)
            xm = work.tile([128, 32], F32, tag="xm")
            nc.vector.tensor_scalar(xm[:], o2b[:], mx[:], None, OP.subtract)
            ew = work.tile([128, 32], F32, tag="ew")
            ssum = work.tile([128, 1], F32, tag="ssum")
            nc.scalar.activation(ew[:], xm[:], AF.Exp, accum_out=ssum[:])
            lns = work.tile([128, 1], F32, tag="lns")
            nc.scalar.activation(lns[:], ssum[:], AF.Ln)
            fin = work.tile([128, 32], F32, tag="fin")
            nc.vector.tensor_scalar(fin[:], xm[:], lns[:], None, OP.subtract)
            nc.sync.dma_start(out_d[w * 128:(w + 1) * 128, :], fin[:])

    nc.compile()
    return nc


_CACHE = {}


def _get_program():
    if "nc" not in _CACHE:
        _CACHE["nc"] = _build_program()
    return _CACHE["nc"]


def _build_timing_program():
    return _build_program(timing=True)


def _host_arrays(inputs):
    x = np.ascontiguousarray(np.asarray(inputs["x"], dtype=np.float32))
    edge_index = np.asarray(inputs["edge_index"])
    W1 = np.asarray(inputs["W1"], dtype=np.float32)
    as1 = np.asarray(inputs["att_src1"], dtype=np.float32)
    ad1 = np.asarray(inputs["att_dst1"], dtype=np.float32)
    b1 = np.asarray(inputs["b1"], dtype=np.float32)
    W2 = np.asarray(inputs["W2"], dtype=np.float32)
    as2 = np.asarray(inputs["att_src2"], dtype=np.float32)
    ad2 = np.asarray(inputs["att_dst2"], dtype=np.float32)
    b2 = np.asarray(inputs["b2"], dtype=np.float32)

    src_idx, dstg_idx, dst_col, node_gid = _preprocess(edge_index)

    xTw = np.zeros((IN_C, TOT), np.float32)
    xTw[:, node_gid] = x.T
    A_src = (W1.reshape(IN_C, HEADS, HID) * as1[None]).sum(-1)
    A_dst = (W1.reshape(IN_C, HEADS, HID) * ad1[None]).sum(-1)
    w1a = np.concatenate([W1, A_src, A_dst], axis=1).astype(np.float32)
    a2s = W2 @ as2[0]
    a2d = W2 @ ad2[0]
    W2A2 = np.concatenate([W2, a2s[:, None], a2d[:, None]], axis=1)  # [256,34]
    w2a = np.concatenate([W2A2[0:128], W2A2[128:256]], axis=1).astype(np.float32)
    iota = np.tile(np.arange(128, dtype=np.float32), (128, 1))
    ident = np.eye(128, dtype=np.float32)
    b1t = np.tile(b1[None, :], (128, 1)).astype(np.float32)
    b2t = np.tile(b2[None, :], (128, 1)).astype(np.float32)

    in_maps = []
    for c in range(NCORES):
        in_maps.append(dict(
            xTw=xTw, w1a=w1a, w2a=w2a, iota=iota, ident=ident,
            b1t=b1t, b2t=b2t,
            sidx=src_idx[c], didx=dstg_idx[c], dcol=dst_col[c],
        ))
    return in_maps, node_gid


def kernel(**inputs):
    in_maps, node_gid = _host_arrays(inputs)
    nc = _get_program()
    res = run_bass_kernel_spmd(nc, in_maps, core_ids=list(range(NCORES)))
    out_full = np.concatenate(
        [np.asarray(res.results[c]["out2"], dtype=np.float32)
         for c in range(NCORES)], axis=0)
    return out_full[node_gid]

